# revision 10
# baseline (speedup 1.0000x reference)
"""Bass/Tile fused attention kernel for nn_AttentionLayer (B=4, S=4096, 256->64).

Sharding: 8 cores = 4 batches x 2 query-halves. Each core gets xT = x[b].T
(bf16, host-transposed, rolled so its own 2048 queries are keys 0..2047),
computes q/k/v projections + flash attention fully on-chip, and writes its
[2048, 64] output slice.

Layouts (per core):
  xT_sb  [128, 2, S]   bf16   x^T, c-tile-major (c = 128*ct + p)
  qT2_sb [128, M]      bf16   q^T duplicated on partition halves (for row-packed QK)
  kT2_sb [128, S/2]    bf16   k^T packed: parts 0:64 = even n-tiles, 64:128 = odd
  v_sb   [128, NT*65]  bf16   v natural per n-tile + ones column (AV stationary),
                              projected directly (lhsT = xT n-tile slice)

Attention (per 512-query chunk): for each n-tile pair j: two row-packed
K=64 matmuls -> scores^T [128, 1024] psum; exp alternates between ACT
(exp(s/8) -> bf16) and DVE (single-pass Schraudolph fast-exp -> i32 tile
whose f32 bitcast feeds AV directly); two AV matmuls accumulate [65, 512]
psum (row 64 = softmax denominator l).
Epilogue: PE "transpose" by R = [[I_64, 0], [bv^T, 1]] folds the bv bias in:
pt = ob^T @ R -> [128, 65] with pt[:,0:64] = z_unnorm + l*bv, pt[:,64] = l;
DVE reciprocal + scale -> z.
"""

import numpy as np
import concourse.bass as bass
import concourse.mybir as mybir
import concourse.tile as tile
from concourse.masks import make_identity

BF16 = mybir.dt.bfloat16
F32 = mybir.dt.float32
AF = mybir.ActivationFunctionType


MAX_WAITS = 1  # this image's walrus allows a single sem wait on most instructions
AV_SPLIT = False  # 2-way K-split AV matmuls (concurrent row tiles)


def _max_waits(inst):
    return MAX_WAITS


def split_excess_waits(nc):
    """Move excess sem-waits from any instruction onto same-engine NOPs
    inserted immediately before it (walrus wait-slot limit workaround)."""
    for f in nc.m.functions:
        for bb in f.blocks:
            insts = list(bb.instructions)
            out, n_new = [], 0
            for inst in insts:
                mw = _max_waits(inst)
                si = inst.sync_info
                waits = list(si.on_wait) if si and si.on_wait else []
                if len(waits) > mw:
                    excess = waits[: len(waits) - mw]
                    keep = waits[len(waits) - mw :]
                    for i in range(0, len(excess), MAX_WAITS):
                        nop = mybir.InstNoOp(
                            name=f"{inst.name}-wsplit{i}", ins=[], outs=[]
                        )
                        nop.engine = inst.engine
                        nop.sync_info = mybir.SyncInfo(
                            on_wait=excess[i : i + MAX_WAITS], on_update=[]
                        )
                        nc.register_instruction(nop, overwrite=True)
                        out.append(nop)
                        n_new += 1
                    inst.sync_info = mybir.SyncInfo(
                        on_wait=keep, on_update=si.on_update
                    )
                out.append(inst)
            if n_new:
                bb.instructions = out


def _ldw_sig(ap_str, tile_position, perf_mode, is_transpose):
    return (ap_str, tile_position, perf_mode, is_transpose)


def _ldw_rect(inst, w):
    tp = inst.tile_position or (0, 0)
    rows = w.ap[0][1]
    cols = 1
    for d in list(w.ap)[1:]:
        cols *= d[1]
    return (tp[0], tp[1], rows, cols)


def dedupe_ldweights(nc):
    """Drop InstLdweights whose weights are already resident in the targeted
    PE-array rectangle (Tile emits one LDW per matmul unconditionally).
    Converted to NOPs to preserve semaphore waits/updates. Tracks (row, col)
    rectangles: loads to disjoint row/col groups don't clobber each other."""
    for f in nc.m.functions:
        for bb in f.blocks:
            insts = list(bb.instructions)
            state = {}  # (row_base, col_base) -> (rows, cols, sig)
            changed = False

            def invalidate(rect):
                rb, cb, rn, cn = rect
                for key in list(state):
                    b_rb, b_cb = key
                    b_rn, b_cn = state[key][0], state[key][1]
                    if (
                        b_rb < rb + rn
                        and rb < b_rb + b_rn
                        and b_cb < cb + cn
                        and cb < b_cb + b_cn
                    ):
                        del state[key]

            out = []
            for inst in insts:
                tn = type(inst).__name__
                if tn == "InstLdweights":
                    w = inst.ins[0]
                    rect = _ldw_rect(inst, w)
                    sig = _ldw_sig(
                        str(w), inst.tile_position, inst.perf_mode, inst.is_transpose
                    )
                    key = (rect[0], rect[1])
                    if state.get(key) == (rect[2], rect[3], sig):
                        nop = mybir.InstNoOp(
                            name=f"{inst.name}-ldwdrop", ins=[], outs=[]
                        )
                        nop.engine = inst.engine
                        if inst.sync_info:
                            nop.sync_info = inst.sync_info
                        nc.register_instruction(nop, overwrite=True)
                        out.append(nop)
                        changed = True
                        continue
                    invalidate(rect)
                    state[key] = (rect[2], rect[3], sig)
                elif tn == "InstMatmult":
                    w = inst.ins[-1]
                    rect = _ldw_rect(inst, w)
                    sig = _ldw_sig(
                        str(w), inst.tile_position, inst.perf_mode, inst.is_transpose
                    )
                    key = (rect[0], rect[1])
                    if state.get(key) != (rect[2], rect[3], sig):
                        # self-loading matmul: it loads its own weights
                        invalidate(rect)
                        state[key] = (rect[2], rect[3], sig)
                out.append(inst)
            if changed:
                bb.instructions = out


def apply_tile_patch():
    """Patch TileContext to split >MAX_WAITS sem-waits (incl. final drain),
    and dedupe redundant LDWEIGHTS."""
    import concourse.tile as _tile

    def _patched(self, tick_clock, wait_clock):
        carrier = self.nc.sync.nop(nofuse=True)
        wait_clock.add_sem_waits(
            carrier.ins, _tile.ScopedClock({None: tick_clock.global_clock})
        )
        si = carrier.ins.sync_info
        waits = list(si.on_wait) if si and si.on_wait else []
        if len(waits) > 1:
            carrier.ins.sync_info = mybir.SyncInfo(
                on_wait=waits[:1], on_update=si.on_update
            )
            for w in waits[1:]:
                extra = self.nc.sync.nop(nofuse=True)
                extra.ins.sync_info = mybir.SyncInfo(on_wait=[w], on_update=[])
        self.nc.sync.drain()
        self.nc.all_engine_barrier()
        assert self.sems is not None
        popped = self.nc._tile_sem_poison_stack.pop()
        assert popped is self._sem_poison
        self.nc.clear_and_free_semaphores(list(self.sems.allocated().values()))
        self.nc.all_engine_barrier()
        dedupe_ldweights(self.nc)
        split_excess_waits(self.nc)

    _tile.TileContext._drain_and_barrier = _patched


def build_graph(S=4096, M=2048, DIN=256, DOUT=64, scale=0.125, dve_exp_every=2):
    """One NeuronCore's graph: M queries attend over S keys."""
    assert DIN == 256 and DOUT == 64
    NT = S // 128          # n-tiles (keys)
    NP = NT // 2           # n-tile pairs
    CH = min(512, M)       # query chunk per PSUM bank
    NCH = M // CH          # chunks
    VBLK = 65              # v_sb per-tile block stride (64 v cols + ones col)
    nc = bass.Bass()

    xT_ext = nc.declare_dram_parameter("xT", [DIN, S], BF16, isOutput=False)
    w_ext = {
        w: nc.declare_dram_parameter(w, [DIN, DOUT], BF16, isOutput=False)
        for w in ("Wq", "Wk", "Wv")
    }
    b_ext = {
        b: nc.declare_dram_parameter(b, [128, 1], F32, isOutput=False)
        for b in ("bq2", "bk2")
    }
    rmat_ext = nc.declare_dram_parameter("Rmat", [DOUT + 1, DOUT + 1], BF16, isOutput=False)
    out_ext = nc.declare_dram_parameter("out", [M, DOUT], F32, isOutput=True)

    # Schraudolph fast-exp in bf16 bit-space (bf16 = top 16 bits of f32):
    # exp(x*scale) ~= bitcast_bf16(i16(A*x + B)), A = 2^7/ln2*scale,
    # B = 127*2^7 - 486411/2^16 (the classic f32 bias scaled down).
    A_C = float((1 << 7) / np.log(2.0) * scale)
    B_C = float(127.0 * (1 << 7) - 486411.0 / 65536.0)

    with tile.TileContext(nc) as tc:
        # PSUM budget (8 banks), all pools coexisting so attention never
        # waits on projection-bank recycling: spool 2x2 + opool 1 + ptpool 1
        # + ppool 1 + vpool 1 = 8.
        with (
            tc.tile_pool(name="singles", bufs=1) as singles,
            tc.tile_pool(name="sb_small", bufs=4) as sb_small,
            tc.tile_pool(name="spsum", bufs=2, space="PSUM") as spool,
            tc.tile_pool(name="opsum", bufs=1, space="PSUM") as opool,
            tc.tile_pool(name="ptpsum", bufs=1, space="PSUM") as ptpool,
            tc.tile_pool(name="ppsum", bufs=1, space="PSUM") as ppool,
            tc.tile_pool(name="vpsum", bufs=1, space="PSUM") as vpool,
            tc.tile_pool(name="pexp", bufs=3) as ppexp,
            tc.tile_pool(name="oout", bufs=2) as oout,
        ):
            # ---- load inputs: small weights/biases/R first (scalar queue) so
            # projections are never blocked behind the big x transfers; x in
            # chunks 0,1 on sync queue and 2,3 on scalar so chunk 0 lands early.
            w_sb = {}
            for w in ("Wq", "Wk", "Wv"):
                w_sb[w] = singles.tile([128, 2, DOUT], BF16, tag=w, name=w + "_sb")
                nc.scalar.dma_start(
                    out=w_sb[w], in_=w_ext[w].rearrange("(c p) d -> p c d", p=128)
                )
            b_sb = {}
            for b in ("bq2", "bk2"):
                b_sb[b] = singles.tile([128, 1], F32, tag=b, name=b + "_sb")
                nc.scalar.dma_start(out=b_sb[b], in_=b_ext[b][:])
            xT_sb = singles.tile([128, 2, S], BF16)
            # 8 chunks alternating between the two HWDGE rings so both pull
            # x concurrently and low-n chunks (needed first) land earliest
            NDC = max(1, S // 512)
            for dchunk in range(NDC):
                n0 = dchunk * (S // NDC)
                n1 = n0 + S // NDC
                eng = nc.sync if dchunk % 2 == 0 else nc.scalar
                eng.dma_start(
                    out=xT_sb[:, :, n0:n1],
                    in_=xT_ext.rearrange("(c p) n -> p c n", p=128)[:, :, n0:n1],
                )
            # R matrix for the bias-folding epilogue transpose, built on host:
            # R[0:64,0:64] = I, R[64,0:64] = bv, R[64,64] = 1, R[0:64,64] = 0
            Rm = singles.tile([VBLK, VBLK], BF16, tag="Rm")
            nc.scalar.dma_start(out=Rm, in_=rmat_ext[:])

            prev_pe = [None]

            def chain(bi):
                # serialize PE matmuls in emission order so same-weights runs
                # stay adjacent (LDW dedup) and pipelining is stable
                if prev_pe[0] is not None:
                    tile.add_dep_helper(
                        bi.ins, prev_pe[0].ins, sync=False, reason="pe-order"
                    )
                prev_pe[0] = bi

            # ---- PE warmup: dummy matmuls on a memset tile while input DMA
            # flies, so the HAM clock gate opens before real work ----
            warm_sb = singles.tile([128, 512], BF16, tag="warm")
            nc.gpsimd.memset(warm_sb, 0.25)
            wp = ppool.tile([128, 512], F32, tag="proj", name="warm")
            for _ in range(4):
                chain(
                    nc.tensor.matmul(
                        wp, lhsT=warm_sb[:, 0:128], rhs=warm_sb[:, 0:512],
                        start=True, stop=True,
                    )
                )

            qT2_sb = singles.tile([128, M], BF16, tag="qT2")
            kT2_sb = singles.tile([128, S // 2], BF16, tag="kT2")
            v_sb = singles.tile([128, NT * VBLK], BF16, tag="vsb")
            nc.gpsimd.memset(v_sb, 1.0)

            # ---- projections in 512-col quarters (1 PSUM bank each).
            # Order: q0, k0, v0 unlock chunk-0 attention after ~2 quarters ----
            PQ = min(512, M)           # q quarter cols
            KQ = min(512, S // 2)      # k quarter cols (packed)
            TPQ = KQ // 128            # n-tile pairs per k quarter

            def emit_proj_q(qi):
                ps = ppool.tile([128, PQ], F32, tag="proj", name=f"psq_{qi}")
                for cg in range(2):
                    for c in range(2):
                        nc.tensor.matmul(
                            ps[64 * cg : 64 * cg + 64, :],
                            lhsT=w_sb["Wq"][:, c, :],
                            rhs=xT_sb[:, c, PQ * qi : PQ * qi + PQ],
                            start=(c == 0),
                            stop=(c == 1),
                            tile_position=(0, 64 * cg),
                        )
                nc.scalar.add(qT2_sb[:, PQ * qi : PQ * qi + PQ], ps, b_sb["bq2"])

            def emit_proj_k(qi):
                ps = ppool.tile([128, KQ], F32, tag="proj", name=f"psk_{qi}")
                for cg in range(2):  # 0 = even n-tiles, 1 = odd
                    for c in range(2):
                        xv = xT_sb[:, c, :].rearrange(
                            "p (u two j) -> p u two j", two=2, j=128
                        )
                        nc.tensor.matmul(
                            ps[64 * cg : 64 * cg + 64, :],
                            lhsT=w_sb["Wk"][:, c, :],
                            rhs=xv[:, TPQ * qi : TPQ * qi + TPQ, cg, :],
                            start=(c == 0),
                            stop=(c == 1),
                            tile_position=(0, 64 * cg),
                        )
                nc.scalar.add(kT2_sb[:, KQ * qi : KQ * qi + KQ], ps, b_sb["bk2"])

            def emit_proj_vnat_group(g):
                # v natural [n, d] for n-tiles 4g..4g+3: stationary xT slices,
                # moving Wv; one batched copy-out. No bias (folded via Rm).
                ps = vpool.tile([128, 4, DOUT], F32, tag="vnat", name=f"psv_{g}")
                for i in range(4):
                    nt = 4 * g + i
                    for c in range(2):
                        nc.tensor.matmul(
                            ps[:, i, :],
                            lhsT=xT_sb[:, c, 128 * nt : 128 * nt + 128],
                            rhs=w_sb["Wv"][:, c, :],
                            start=(c == 0),
                            stop=(c == 1),
                        )
                dst = v_sb.rearrange("p (b r) -> p b r", r=VBLK)[
                    :, 4 * g : 4 * g + 4, 0:64
                ]
                nc.vector.tensor_copy(dst, ps)

            NKQ = (S // 2) // KQ
            emit_proj_q(0)
            for qi in range(NKQ):
                emit_proj_k(qi)
                for g in range(2 * qi, 2 * qi + 2):
                    emit_proj_vnat_group(g)

            # ---- attention: chunk-outer, pair-inner; exp alternates ACT/DVE;
            # QK/exp of pair j+1 are emitted before AV of pair j so the PE has
            # work while exp(j) runs ----
            if True:
                pending_steps = []

                def emit_proj_q_late(qi):
                    # q quarters 1-3 are only needed from chunk 1 on; emit
                    # them inside chunk 0's pair stream on the proj psum pool
                    ps = ppool.tile([128, PQ], F32, tag="proj", name=f"qps_{qi}")
                    for cg in range(2):
                        for c in range(2):
                            nc.tensor.matmul(
                                ps[64 * cg : 64 * cg + 64, :],
                                lhsT=w_sb["Wq"][:, c, :],
                                rhs=xT_sb[:, c, PQ * qi : PQ * qi + PQ],
                                start=(c == 0),
                                stop=(c == 1),
                                tile_position=(0, 64 * cg),
                            )
                    nc.scalar.add(
                        qT2_sb[:, PQ * qi : PQ * qi + PQ], ps,
                        b_sb["bq2"],
                    )

                proj_work = [
                    (lambda qi=qi: emit_proj_q_late(qi))
                    for qi in range(1, M // PQ)
                ]
                for mc in range(NCH):
                    mlo = CH * mc
                    po = opool.tile([VBLK, CH], F32, tag="po", name=f"po_{mc}")
                    s_t = [None] * NP
                    p_t = [None] * NP

                    def emit_qk_exp(j):
                        s = spool.tile(
                            [128, 2 * CH], F32, tag="s", name=f"s_{mc}_{j}"
                        )
                        for half in range(2):
                            nc.tensor.matmul(
                                s[:, CH * half : CH * half + CH],
                                lhsT=kT2_sb[
                                    64 * half : 64 * half + 64,
                                    128 * j : 128 * j + 128,
                                ],
                                rhs=qT2_sb[
                                    64 * half : 64 * half + 64, mlo : mlo + CH
                                ],
                                start=True,
                                stop=True,
                                tile_position=(64 * half, 0),
                            )
                        s_t[j] = s
                        di = mc * NP + j
                        if dve_exp_every and di % 16 in (1, 3, 5, 7, 9, 11, 13):
                            # Schraudolph fast-exp on the (otherwise idle) DVE;
                            # AV reads the bf16 bitcast directly (no copy).
                            i16 = ppexp.tile(
                                [128, 2 * CH], mybir.dt.int16, tag="pi",
                                name=f"pi_{mc}_{j}",
                            )
                            nc.vector.tensor_scalar(
                                i16, s, A_C, B_C,
                                op0=mybir.AluOpType.mult,
                                op1=mybir.AluOpType.add,
                            )
                            p_t[j] = i16.bitcast(BF16)
                        else:
                            p = ppexp.tile(
                                [128, 2 * CH], BF16, tag="p", name=f"p_{mc}_{j}"
                            )
                            nc.scalar.activation(p, s, AF.Exp, scale=scale)
                            p_t[j] = p

                    def emit_av(j):
                        # AV_SPLIT: 2-way K-split (rows 0:64 / 64:128) per
                        # n-tile so each half's LDWEIGHTS overlaps the other
                        # half's matmul; plain K=128 matmuls otherwise
                        for half in range(2):
                            vt = v_sb[
                                :,
                                VBLK * (2 * j + half) : VBLK * (2 * j + half)
                                + VBLK,
                            ]
                            pt_ = p_t[j][:, CH * half : CH * half + CH]
                            if AV_SPLIT:
                                for ks in range(2):
                                    nc.tensor.matmul(
                                        po,
                                        lhsT=vt[64 * ks : 64 * ks + 64, :],
                                        rhs=pt_[64 * ks : 64 * ks + 64, :],
                                        start=(j == 0 and half == 0 and ks == 0),
                                        stop=(
                                            j == NP - 1 and half == 1 and ks == 1
                                        ),
                                        tile_position=(64 * ks, 0),
                                    )
                            else:
                                nc.tensor.matmul(
                                    po,
                                    lhsT=vt,
                                    rhs=pt_,
                                    start=(j == 0 and half == 0),
                                    stop=(j == NP - 1 and half == 1),
                                )
                        s_t[j] = None
                        p_t[j] = None

                    emit_qk_exp(0)
                    emit_qk_exp(1)
                    for j in range(NP):
                        if proj_work:
                            proj_work.pop(0)()
                        if pending_steps:
                            pending_steps.pop(0)()
                        if j + 2 < NP:
                            emit_qk_exp(j + 2)
                        emit_av(j)

                    # epilogue: matmul-by-R (adds bv), divide by l, store.
                    # Emitted as one step per pair of the NEXT chunk so the PE
                    # keeps streaming QKs across the chunk boundary and the
                    # single-buffer pt ring never stalls it. bf16 operands:
                    # plain matmul (transpose-mode ignores rhs content, so R
                    # must go through the regular path) and bf16 avoids the
                    # slow fp32 LOW/HIGH double-pass.
                    cell = {}

                    def step_obcopy(mc=mc, po=po):
                        ob = oout.tile([VBLK, CH], BF16, tag="ob", name=f"ob_{mc}")
                        nc.scalar.copy(ob, po)
                        cell["ob"] = ob
                        cell["zb"] = oout.tile(
                            [128, CH // 128, 64], F32, tag="zb", name=f"zb_{mc}"
                        )

                    def step_t(t, mc=mc):
                        # strided ob slice: pt partition p = query mlo+4p+t,
                        # so each out-DMA descriptor covers 4 contiguous rows
                        ob, zb = cell["ob"], cell["zb"]
                        pt = ptpool.tile(
                            [128, VBLK], F32, tag="pt", name=f"zt_{mc}_{t}"
                        )
                        obv = ob.rearrange("v (f four) -> v f four", four=4)
                        nc.tensor.matmul(
                            pt,
                            lhsT=obv[:, :, t],
                            rhs=Rm,
                            start=True,
                            stop=True,
                        )
                        r = sb_small.tile([128, 1], F32, tag="r", name="r_t")
                        nc.vector.reciprocal(r, pt[:, 64:65])
                        nc.vector.tensor_scalar_mul(zb[:, t, :], pt[:, 0:64], r)

                    def step_dma(mlo=mlo):
                        nc.sync.dma_start(
                            out=out_ext[mlo : mlo + CH, :].rearrange(
                                "(p four) d -> p four d", four=4
                            ),
                            in_=cell.pop("zb"),
                        )
                        cell.pop("ob")

                    pending_steps = [step_obcopy]
                    pending_steps += [
                        (lambda t=t: step_t(t)) for t in range(CH // 128)
                    ]
                    pending_steps.append(step_dma)
                for st in pending_steps:
                    st()
                pending_steps = []
    return nc


def _make_rmat(bv):
    import ml_dtypes

    d = bv.shape[0]
    R = np.zeros((d + 1, d + 1), np.float32)
    R[:d, :d] = np.eye(d, dtype=np.float32)
    R[d, :d] = bv
    R[d, d] = 1.0
    return R.astype(ml_dtypes.bfloat16)


def make_in_maps(x, Wq, bq, Wk, bk, Wv, bv, n_cores=8):
    """Host-side sharding: core i handles batch i//2, query half i%2."""
    import ml_dtypes

    bf16 = ml_dtypes.bfloat16
    B, S, DIN = x.shape
    M = S // 2
    Ws = {
        "Wq": np.ascontiguousarray(Wq).astype(bf16),
        "Wk": np.ascontiguousarray(Wk).astype(bf16),
        "Wv": np.ascontiguousarray(Wv).astype(bf16),
    }
    bs = {
        "bq2": np.concatenate([bq, bq]).reshape(128, 1).astype(np.float32),
        "bk2": np.concatenate([bk, bk]).reshape(128, 1).astype(np.float32),
        "Rmat": _make_rmat(bv),
    }
    in_maps = []
    for i in range(n_cores):
        b, half = i // 2, i % 2
        xb = np.roll(x[b], -half * M, axis=0)  # own queries first
        xT = np.ascontiguousarray(xb.T).astype(bf16)
        in_maps.append({"xT": xT, **Ws, **bs})
    return in_maps


def assemble_out(results, B=4, S=4096, DOUT=64):
    M = S // 2
    z = np.empty((B, S, DOUT), np.float32)
    for i, res in enumerate(results):
        b, half = i // 2, i % 2
        z[b, half * M : (half + 1) * M] = res["out"]
    return z


_GRAPH_CACHE = {}


def kernel(x, Wq, bq, Wk, bk, Wv, bv):
    """Full-input entry point: shards across 8 NeuronCores (batch x
    query-half), runs the Bass kernel SPMD, gathers the full [B, S, 64]
    float32 output."""
    from concourse.bass_utils import run_bass_kernel_spmd

    apply_tile_patch()
    x = np.asarray(x, dtype=np.float32)
    Wq, bq = np.asarray(Wq, np.float32), np.asarray(bq, np.float32)
    Wk, bk = np.asarray(Wk, np.float32), np.asarray(bk, np.float32)
    Wv, bv = np.asarray(Wv, np.float32), np.asarray(bv, np.float32)
    B, S, DIN = x.shape
    DOUT = Wq.shape[1]
    key = (S, DIN, DOUT)
    if key not in _GRAPH_CACHE:
        _GRAPH_CACHE[key] = build_graph(
            S=S, M=S // 2, DIN=DIN, DOUT=DOUT, scale=1.0 / float(np.sqrt(DOUT))
        )
    nc = _GRAPH_CACHE[key]
    in_maps = make_in_maps(x, Wq, bq, Wk, bk, Wv, bv, n_cores=2 * B)
    res = run_bass_kernel_spmd(nc, in_maps, list(range(2 * B)))
    return assemble_out(res.results, B=B, S=S, DOUT=DOUT)



# revision 13
# speedup vs baseline: 1.2326x; 1.2326x over previous
"""Bass/Tile fused attention kernel for nn_AttentionLayer (B=4, S=4096, 256->64).

Sharding: 8 cores = 4 batches x 2 query-halves. Each core gets xT = x[b].T
(bf16, host-transposed, rolled so its own 2048 queries are keys 0..2047),
computes q/k/v projections + flash attention fully on-chip, and writes its
[2048, 64] output slice.

Layouts (per core):
  xT_sb  [128, 2, S]   bf16   x^T, c-tile-major (c = 128*ct + p)
  qT2_sb [128, M]      bf16   q^T duplicated on partition halves (for row-packed QK)
  kT2_sb [128, S/2]    bf16   k^T packed: parts 0:64 = even n-tiles, 64:128 = odd
  v_sb   [128, NT*65]  bf16   v natural per n-tile + ones column (AV stationary),
                              projected directly (lhsT = xT n-tile slice)

Attention (per 512-query chunk): for each n-tile pair j: two row-packed
K=64 matmuls -> scores^T [128, 1024] psum; exp alternates between ACT
(exp(s/8) -> bf16) and DVE (single-pass Schraudolph fast-exp -> i32 tile
whose f32 bitcast feeds AV directly); two AV matmuls accumulate [65, 512]
psum (row 64 = softmax denominator l).
Epilogue: PE "transpose" by R = [[I_64, 0], [bv^T, 1]] folds the bv bias in:
pt = ob^T @ R -> [128, 65] with pt[:,0:64] = z_unnorm + l*bv, pt[:,64] = l;
DVE reciprocal + scale -> z.
"""

import numpy as np
import concourse.bass as bass
import concourse.mybir as mybir
import concourse.tile as tile
from concourse.masks import make_identity

BF16 = mybir.dt.bfloat16
F32 = mybir.dt.float32
AF = mybir.ActivationFunctionType


MAX_WAITS = 1  # this image's walrus allows a single sem wait on most instructions
AV_SPLIT = False  # 2-way K-split AV matmuls (concurrent row tiles)


def _max_waits(inst):
    return MAX_WAITS


def split_excess_waits(nc):
    """Move excess sem-waits from any instruction onto same-engine NOPs
    inserted immediately before it (walrus wait-slot limit workaround)."""
    for f in nc.m.functions:
        for bb in f.blocks:
            insts = list(bb.instructions)
            out, n_new = [], 0
            for inst in insts:
                mw = _max_waits(inst)
                si = inst.sync_info
                waits = list(si.on_wait) if si and si.on_wait else []
                if len(waits) > mw:
                    excess = waits[: len(waits) - mw]
                    keep = waits[len(waits) - mw :]
                    for i in range(0, len(excess), MAX_WAITS):
                        nop = mybir.InstNoOp(
                            name=f"{inst.name}-wsplit{i}", ins=[], outs=[]
                        )
                        nop.engine = inst.engine
                        nop.sync_info = mybir.SyncInfo(
                            on_wait=excess[i : i + MAX_WAITS], on_update=[]
                        )
                        nc.register_instruction(nop, overwrite=True)
                        out.append(nop)
                        n_new += 1
                    inst.sync_info = mybir.SyncInfo(
                        on_wait=keep, on_update=si.on_update
                    )
                out.append(inst)
            if n_new:
                bb.instructions = out


def _ldw_sig(ap_str, tile_position, perf_mode, is_transpose):
    return (ap_str, tile_position, perf_mode, is_transpose)


def _ldw_rect(inst, w):
    tp = inst.tile_position or (0, 0)
    rows = w.ap[0][1]
    cols = 1
    for d in list(w.ap)[1:]:
        cols *= d[1]
    return (tp[0], tp[1], rows, cols)


def dedupe_ldweights(nc):
    """Drop InstLdweights whose weights are already resident in the targeted
    PE-array rectangle (Tile emits one LDW per matmul unconditionally).
    Converted to NOPs to preserve semaphore waits/updates. Tracks (row, col)
    rectangles: loads to disjoint row/col groups don't clobber each other."""
    for f in nc.m.functions:
        for bb in f.blocks:
            insts = list(bb.instructions)
            state = {}  # (row_base, col_base) -> (rows, cols, sig)
            changed = False

            def invalidate(rect):
                rb, cb, rn, cn = rect
                for key in list(state):
                    b_rb, b_cb = key
                    b_rn, b_cn = state[key][0], state[key][1]
                    if (
                        b_rb < rb + rn
                        and rb < b_rb + b_rn
                        and b_cb < cb + cn
                        and cb < b_cb + b_cn
                    ):
                        del state[key]

            out = []
            for inst in insts:
                tn = type(inst).__name__
                if tn == "InstLdweights":
                    w = inst.ins[0]
                    rect = _ldw_rect(inst, w)
                    sig = _ldw_sig(
                        str(w), inst.tile_position, inst.perf_mode, inst.is_transpose
                    )
                    key = (rect[0], rect[1])
                    if state.get(key) == (rect[2], rect[3], sig):
                        nop = mybir.InstNoOp(
                            name=f"{inst.name}-ldwdrop", ins=[], outs=[]
                        )
                        nop.engine = inst.engine
                        if inst.sync_info:
                            nop.sync_info = inst.sync_info
                        nc.register_instruction(nop, overwrite=True)
                        out.append(nop)
                        changed = True
                        continue
                    invalidate(rect)
                    state[key] = (rect[2], rect[3], sig)
                elif tn == "InstMatmult":
                    w = inst.ins[-1]
                    rect = _ldw_rect(inst, w)
                    sig = _ldw_sig(
                        str(w), inst.tile_position, inst.perf_mode, inst.is_transpose
                    )
                    key = (rect[0], rect[1])
                    if state.get(key) != (rect[2], rect[3], sig):
                        # self-loading matmul: it loads its own weights
                        invalidate(rect)
                        state[key] = (rect[2], rect[3], sig)
                out.append(inst)
            if changed:
                bb.instructions = out


def apply_tile_patch():
    """Patch TileContext to split >MAX_WAITS sem-waits (incl. final drain),
    and dedupe redundant LDWEIGHTS."""
    import concourse.tile as _tile

    def _patched(self, tick_clock, wait_clock):
        carrier = self.nc.sync.nop(nofuse=True)
        wait_clock.add_sem_waits(
            carrier.ins, _tile.ScopedClock({None: tick_clock.global_clock})
        )
        si = carrier.ins.sync_info
        waits = list(si.on_wait) if si and si.on_wait else []
        if len(waits) > 1:
            carrier.ins.sync_info = mybir.SyncInfo(
                on_wait=waits[:1], on_update=si.on_update
            )
            for w in waits[1:]:
                extra = self.nc.sync.nop(nofuse=True)
                extra.ins.sync_info = mybir.SyncInfo(on_wait=[w], on_update=[])
        self.nc.sync.drain()
        self.nc.all_engine_barrier()
        assert self.sems is not None
        popped = self.nc._tile_sem_poison_stack.pop()
        assert popped is self._sem_poison
        self.nc.clear_and_free_semaphores(list(self.sems.allocated().values()))
        self.nc.all_engine_barrier()
        dedupe_ldweights(self.nc)
        split_excess_waits(self.nc)

    _tile.TileContext._drain_and_barrier = _patched


def build_graph(S=4096, M=2048, DIN=256, DOUT=64, scale=0.125, dve_exp_every=2):
    """One NeuronCore's graph: M queries attend over S keys."""
    assert DIN == 256 and DOUT == 64
    NT = S // 128          # n-tiles (keys)
    NP = NT // 2           # n-tile pairs
    CH = min(512, M)       # query chunk per PSUM bank
    NCH = M // CH          # chunks
    VBLK = 65              # v_sb per-tile block stride (64 v cols + ones col)
    nc = bass.Bass()

    xT_ext = nc.declare_dram_parameter("xT", [DIN, S], BF16, isOutput=False)
    w_ext = {
        w: nc.declare_dram_parameter(w, [DIN, DOUT], BF16, isOutput=False)
        for w in ("Wq", "Wk", "Wv")
    }
    b_ext = {
        b: nc.declare_dram_parameter(b, [128, 1], F32, isOutput=False)
        for b in ("bq2", "bk2")
    }
    rmat_ext = nc.declare_dram_parameter("Rmat", [DOUT + 1, DOUT + 1], BF16, isOutput=False)
    out_ext = nc.declare_dram_parameter("out", [M, DOUT], F32, isOutput=True)

    # Schraudolph fast-exp in bf16 bit-space (bf16 = top 16 bits of f32):
    # exp(x*scale) ~= bitcast_bf16(i16(A*x + B)), A = 2^7/ln2*scale,
    # B = 127*2^7 - 486411/2^16 (the classic f32 bias scaled down).
    A_C = float((1 << 7) / np.log(2.0) * scale)
    B_C = float(127.0 * (1 << 7) - 486411.0 / 65536.0)

    with tile.TileContext(nc) as tc:
        with (
            tc.tile_pool(name="singles", bufs=1) as singles,
            tc.tile_pool(name="sb_small", bufs=4) as sb_small,
        ):
            # ---- load inputs: small weights/biases/R first (scalar queue) so
            # projections are never blocked behind the big x transfers; x in
            # chunks 0,1 on sync queue and 2,3 on scalar so chunk 0 lands early.
            w_sb = {}
            for w in ("Wq", "Wk", "Wv"):
                w_sb[w] = singles.tile([128, 2, DOUT], BF16, tag=w, name=w + "_sb")
                nc.scalar.dma_start(
                    out=w_sb[w], in_=w_ext[w].rearrange("(c p) d -> p c d", p=128)
                )
            b_sb = {}
            for b in ("bq2", "bk2"):
                b_sb[b] = singles.tile([128, 1], F32, tag=b, name=b + "_sb")
                nc.scalar.dma_start(out=b_sb[b], in_=b_ext[b][:])
            xT_sb = singles.tile([128, 2, S], BF16)
            # 8 chunks alternating between the two HWDGE rings so both pull
            # x concurrently and low-n chunks (needed first) land earliest
            NDC = max(1, S // 512)
            for dchunk in range(NDC):
                n0 = dchunk * (S // NDC)
                n1 = n0 + S // NDC
                eng = nc.sync if dchunk % 2 == 0 else nc.scalar
                eng.dma_start(
                    out=xT_sb[:, :, n0:n1],
                    in_=xT_ext.rearrange("(c p) n -> p c n", p=128)[:, :, n0:n1],
                )
            # R matrix for the bias-folding epilogue transpose, built on host:
            # R[0:64,0:64] = I, R[64,0:64] = bv, R[64,64] = 1, R[0:64,64] = 0
            Rm = singles.tile([VBLK, VBLK], BF16, tag="Rm")
            nc.scalar.dma_start(out=Rm, in_=rmat_ext[:])

            prev_pe = [None]

            def chain(bi):
                # serialize PE matmuls in emission order so same-weights runs
                # stay adjacent (LDW dedup) and pipelining is stable
                if prev_pe[0] is not None:
                    tile.add_dep_helper(
                        bi.ins, prev_pe[0].ins, sync=False, reason="pe-order"
                    )
                prev_pe[0] = bi

            # ---- projection pools: 3 bufs each and warmup inside ppool, so
            # during proj only 6 PSUM banks are ever held; the attention
            # pools (created after proj) land their first bufs on fresh or
            # early-freed banks and the first pairs start ~10us ----
            ppool_cm = tc.tile_pool(name="ppsum", bufs=3, space="PSUM")
            ppool = ppool_cm.__enter__()
            vpool_cm = tc.tile_pool(name="vpsum", bufs=3, space="PSUM")
            vpool = vpool_cm.__enter__()

            # ---- PE warmup: dummy matmuls on a memset tile while input DMA
            # flies, so the HAM clock gate opens before real work ----
            warm_sb = singles.tile([128, 512], BF16, tag="warm")
            nc.gpsimd.memset(warm_sb, 0.25)
            wp = ppool.tile([128, 512], F32, tag="proj", name="warm")
            for _ in range(4):
                chain(
                    nc.tensor.matmul(
                        wp, lhsT=warm_sb[:, 0:128], rhs=warm_sb[:, 0:512],
                        start=True, stop=True,
                    )
                )

            qT2_sb = singles.tile([128, M], BF16, tag="qT2")
            kT2_sb = singles.tile([128, S // 2], BF16, tag="kT2")
            v_sb = singles.tile([128, NT * VBLK], BF16, tag="vsb")
            nc.gpsimd.memset(v_sb, 1.0)

            # ---- projections in 512-col quarters (1 PSUM bank each).
            # Order: q0, k0, v0 unlock chunk-0 attention after ~2 quarters ----
            PQ = min(512, M)           # q quarter cols
            KQ = min(512, S // 2)      # k quarter cols (packed)
            TPQ = KQ // 128            # n-tile pairs per k quarter

            def emit_proj_q(qi):
                ps = ppool.tile([128, PQ], F32, tag="proj", name=f"psq_{qi}")
                for cg in range(2):
                    for c in range(2):
                        nc.tensor.matmul(
                            ps[64 * cg : 64 * cg + 64, :],
                            lhsT=w_sb["Wq"][:, c, :],
                            rhs=xT_sb[:, c, PQ * qi : PQ * qi + PQ],
                            start=(c == 0),
                            stop=(c == 1),
                            tile_position=(0, 64 * cg),
                        )
                nc.scalar.add(qT2_sb[:, PQ * qi : PQ * qi + PQ], ps, b_sb["bq2"])

            def emit_proj_k(qi):
                ps = ppool.tile([128, KQ], F32, tag="proj", name=f"psk_{qi}")
                for cg in range(2):  # 0 = even n-tiles, 1 = odd
                    for c in range(2):
                        xv = xT_sb[:, c, :].rearrange(
                            "p (u two j) -> p u two j", two=2, j=128
                        )
                        nc.tensor.matmul(
                            ps[64 * cg : 64 * cg + 64, :],
                            lhsT=w_sb["Wk"][:, c, :],
                            rhs=xv[:, TPQ * qi : TPQ * qi + TPQ, cg, :],
                            start=(c == 0),
                            stop=(c == 1),
                            tile_position=(0, 64 * cg),
                        )
                nc.scalar.add(kT2_sb[:, KQ * qi : KQ * qi + KQ], ps, b_sb["bk2"])

            def emit_proj_vnat_group(g):
                # v natural [n, d] for n-tiles 4g..4g+3: stationary xT slices,
                # moving Wv; one batched copy-out. No bias (folded via Rm).
                ps = vpool.tile([128, 4, DOUT], F32, tag="vnat", name=f"psv_{g}")
                for i in range(4):
                    nt = 4 * g + i
                    for c in range(2):
                        nc.tensor.matmul(
                            ps[:, i, :],
                            lhsT=xT_sb[:, c, 128 * nt : 128 * nt + 128],
                            rhs=w_sb["Wv"][:, c, :],
                            start=(c == 0),
                            stop=(c == 1),
                        )
                dst = v_sb.rearrange("p (b r) -> p b r", r=VBLK)[
                    :, 4 * g : 4 * g + 4, 0:64
                ]
                nc.vector.tensor_copy(dst, ps)

            NKQ = (S // 2) // KQ
            emit_proj_q(0)
            for qi in range(NKQ):
                emit_proj_k(qi)
                for g in range(2 * qi, 2 * qi + 2):
                    emit_proj_vnat_group(g)
            vpool_cm.__exit__(None, None, None)
            ppool_cm.__exit__(None, None, None)

            # ---- attention: chunk-outer, pair-inner; exp alternates ACT/DVE;
            # QK/exp of pair j+1 are emitted before AV of pair j so the PE has
            # work while exp(j) runs ----
            with (
                tc.tile_pool(name="spsum", bufs=3, space="PSUM") as spool,
                tc.tile_pool(name="opsum", bufs=1, space="PSUM") as opool,
                tc.tile_pool(name="ptpsum", bufs=1, space="PSUM") as ptpool,
                tc.tile_pool(name="pexp", bufs=3) as ppexp,
                tc.tile_pool(name="oout", bufs=2) as oout,
            ):
                pending_steps = []

                def emit_proj_q_late(qi):
                    # q quarters 1-3 are only needed from chunk 1 on; emit
                    # them inside chunk 0's pair stream, borrowing an s-ring
                    # slot for the PSUM (ppool is closed by now).
                    ps = spool.tile(
                        [128, 2 * CH], F32, tag="s", name=f"qps_{qi}"
                    )
                    for cg in range(2):
                        for c in range(2):
                            nc.tensor.matmul(
                                ps[64 * cg : 64 * cg + 64, 0:PQ],
                                lhsT=w_sb["Wq"][:, c, :],
                                rhs=xT_sb[:, c, PQ * qi : PQ * qi + PQ],
                                start=(c == 0),
                                stop=(c == 1),
                                tile_position=(0, 64 * cg),
                            )
                    nc.scalar.add(
                        qT2_sb[:, PQ * qi : PQ * qi + PQ], ps[:, 0:PQ],
                        b_sb["bq2"],
                    )

                proj_work = [
                    (lambda qi=qi: emit_proj_q_late(qi))
                    for qi in range(1, M // PQ)
                ]
                for mc in range(NCH):
                    mlo = CH * mc
                    po = opool.tile([VBLK, CH], F32, tag="po", name=f"po_{mc}")
                    s_t = [None] * NP
                    p_t = [None] * NP

                    def emit_qk_exp(j):
                        s = spool.tile(
                            [128, 2 * CH], F32, tag="s", name=f"s_{mc}_{j}"
                        )
                        for half in range(2):
                            nc.tensor.matmul(
                                s[:, CH * half : CH * half + CH],
                                lhsT=kT2_sb[
                                    64 * half : 64 * half + 64,
                                    128 * j : 128 * j + 128,
                                ],
                                rhs=qT2_sb[
                                    64 * half : 64 * half + 64, mlo : mlo + CH
                                ],
                                start=True,
                                stop=True,
                                tile_position=(64 * half, 0),
                            )
                        s_t[j] = s
                        di = mc * NP + j
                        if dve_exp_every and di % 16 in (1, 3, 5, 7, 9, 11, 13):
                            # Schraudolph fast-exp on the (otherwise idle) DVE;
                            # AV reads the bf16 bitcast directly (no copy).
                            i16 = ppexp.tile(
                                [128, 2 * CH], mybir.dt.int16, tag="pi",
                                name=f"pi_{mc}_{j}",
                            )
                            nc.vector.tensor_scalar(
                                i16, s, A_C, B_C,
                                op0=mybir.AluOpType.mult,
                                op1=mybir.AluOpType.add,
                            )
                            p_t[j] = i16.bitcast(BF16)
                        else:
                            p = ppexp.tile(
                                [128, 2 * CH], BF16, tag="p", name=f"p_{mc}_{j}"
                            )
                            nc.scalar.activation(p, s, AF.Exp, scale=scale)
                            p_t[j] = p

                    def emit_av(j):
                        # AV_SPLIT: 2-way K-split (rows 0:64 / 64:128) per
                        # n-tile so each half's LDWEIGHTS overlaps the other
                        # half's matmul; plain K=128 matmuls otherwise
                        for half in range(2):
                            vt = v_sb[
                                :,
                                VBLK * (2 * j + half) : VBLK * (2 * j + half)
                                + VBLK,
                            ]
                            pt_ = p_t[j][:, CH * half : CH * half + CH]
                            if AV_SPLIT:
                                for ks in range(2):
                                    nc.tensor.matmul(
                                        po,
                                        lhsT=vt[64 * ks : 64 * ks + 64, :],
                                        rhs=pt_[64 * ks : 64 * ks + 64, :],
                                        start=(j == 0 and half == 0 and ks == 0),
                                        stop=(
                                            j == NP - 1 and half == 1 and ks == 1
                                        ),
                                        tile_position=(64 * ks, 0),
                                    )
                            else:
                                nc.tensor.matmul(
                                    po,
                                    lhsT=vt,
                                    rhs=pt_,
                                    start=(j == 0 and half == 0),
                                    stop=(j == NP - 1 and half == 1),
                                )
                        s_t[j] = None
                        p_t[j] = None

                    emit_qk_exp(0)
                    emit_qk_exp(1)
                    for j in range(NP):
                        if proj_work:
                            proj_work.pop(0)()
                        if pending_steps:
                            pending_steps.pop(0)()
                        if j + 2 < NP:
                            emit_qk_exp(j + 2)
                        emit_av(j)

                    # epilogue: matmul-by-R (adds bv), divide by l, store.
                    # Emitted as one step per pair of the NEXT chunk so the PE
                    # keeps streaming QKs across the chunk boundary and the
                    # single-buffer pt ring never stalls it. bf16 operands:
                    # plain matmul (transpose-mode ignores rhs content, so R
                    # must go through the regular path) and bf16 avoids the
                    # slow fp32 LOW/HIGH double-pass.
                    cell = {}

                    def step_obcopy(mc=mc, po=po):
                        ob = oout.tile([VBLK, CH], BF16, tag="ob", name=f"ob_{mc}")
                        nc.scalar.copy(ob, po)
                        cell["ob"] = ob
                        cell["zb"] = oout.tile(
                            [128, CH // 128, 64], F32, tag="zb", name=f"zb_{mc}"
                        )

                    def step_t(t, mc=mc):
                        # strided ob slice: pt partition p = query mlo+4p+t,
                        # so each out-DMA descriptor covers 4 contiguous rows
                        ob, zb = cell["ob"], cell["zb"]
                        pt = ptpool.tile(
                            [128, VBLK], F32, tag="pt", name=f"zt_{mc}_{t}"
                        )
                        obv = ob.rearrange("v (f four) -> v f four", four=4)
                        nc.tensor.matmul(
                            pt,
                            lhsT=obv[:, :, t],
                            rhs=Rm,
                            start=True,
                            stop=True,
                        )
                        r = sb_small.tile([128, 1], F32, tag="r", name="r_t")
                        nc.vector.reciprocal(r, pt[:, 64:65])
                        nc.vector.tensor_scalar_mul(zb[:, t, :], pt[:, 0:64], r)

                    def step_dma(mlo=mlo):
                        nc.sync.dma_start(
                            out=out_ext[mlo : mlo + CH, :].rearrange(
                                "(p four) d -> p four d", four=4
                            ),
                            in_=cell.pop("zb"),
                        )
                        cell.pop("ob")

                    pending_steps = [step_obcopy]
                    pending_steps += [
                        (lambda t=t: step_t(t)) for t in range(CH // 128)
                    ]
                    pending_steps.append(step_dma)
                for st in pending_steps:
                    st()
                pending_steps = []
    return nc


def _make_rmat(bv):
    import ml_dtypes

    d = bv.shape[0]
    R = np.zeros((d + 1, d + 1), np.float32)
    R[:d, :d] = np.eye(d, dtype=np.float32)
    R[d, :d] = bv
    R[d, d] = 1.0
    return R.astype(ml_dtypes.bfloat16)


def make_in_maps(x, Wq, bq, Wk, bk, Wv, bv, n_cores=8):
    """Host-side sharding: core i handles batch i//2, query half i%2."""
    import ml_dtypes

    bf16 = ml_dtypes.bfloat16
    B, S, DIN = x.shape
    M = S // 2
    Ws = {
        "Wq": np.ascontiguousarray(Wq).astype(bf16),
        "Wk": np.ascontiguousarray(Wk).astype(bf16),
        "Wv": np.ascontiguousarray(Wv).astype(bf16),
    }
    bs = {
        "bq2": np.concatenate([bq, bq]).reshape(128, 1).astype(np.float32),
        "bk2": np.concatenate([bk, bk]).reshape(128, 1).astype(np.float32),
        "Rmat": _make_rmat(bv),
    }
    in_maps = []
    for i in range(n_cores):
        b, half = i // 2, i % 2
        xb = np.roll(x[b], -half * M, axis=0)  # own queries first
        xT = np.ascontiguousarray(xb.T).astype(bf16)
        in_maps.append({"xT": xT, **Ws, **bs})
    return in_maps


def assemble_out(results, B=4, S=4096, DOUT=64):
    M = S // 2
    z = np.empty((B, S, DOUT), np.float32)
    for i, res in enumerate(results):
        b, half = i // 2, i % 2
        z[b, half * M : (half + 1) * M] = res["out"]
    return z


_GRAPH_CACHE = {}


def kernel(x, Wq, bq, Wk, bk, Wv, bv):
    """Full-input entry point: shards across 8 NeuronCores (batch x
    query-half), runs the Bass kernel SPMD, gathers the full [B, S, 64]
    float32 output."""
    from concourse.bass_utils import run_bass_kernel_spmd

    apply_tile_patch()
    x = np.asarray(x, dtype=np.float32)
    Wq, bq = np.asarray(Wq, np.float32), np.asarray(bq, np.float32)
    Wk, bk = np.asarray(Wk, np.float32), np.asarray(bk, np.float32)
    Wv, bv = np.asarray(Wv, np.float32), np.asarray(bv, np.float32)
    B, S, DIN = x.shape
    DOUT = Wq.shape[1]
    key = (S, DIN, DOUT)
    if key not in _GRAPH_CACHE:
        _GRAPH_CACHE[key] = build_graph(
            S=S, M=S // 2, DIN=DIN, DOUT=DOUT, scale=1.0 / float(np.sqrt(DOUT))
        )
    nc = _GRAPH_CACHE[key]
    in_maps = make_in_maps(x, Wq, bq, Wk, bk, Wv, bv, n_cores=2 * B)
    res = run_bass_kernel_spmd(nc, in_maps, list(range(2 * B)))
    return assemble_out(res.results, B=B, S=S, DOUT=DOUT)

